# revision 13
# baseline (speedup 1.0000x reference)
"""Multi-head attention (B=2, S=2048, D=1024, H=16) on 8 TRN2 NeuronCores.

Sharding: tensor-parallel over heads. Each core owns 2 heads: Wq/Wk/Wv are
split column-wise (128 cols per core), Wo row-wise (128 rows per core). Each
core computes its heads' attention and a partial output projection
y_c = O_c @ Wo_c; the host sums the 8 partials and adds bo.

Per-core device pipeline (all fp32):
  - host passes xT_aug [1152, 4096]: x.T with a ones row (row 1024) so the
    projection biases ride in the matmul as an extra contraction row; rows
    1025.. are zero padding to a multiple of 128.
  - QT/KT/VT projections: out[d2=128, s] = Wq_aug.T @ xT_aug per 512-col
    chunk, accumulating 9 K-tiles in PSUM. Scale 1/sqrt(dk) is folded into
    Wq/bq on the host.
  - V transposed on PE (identity matmul) to natural [k, d] layout with a
    ones column appended per head -> PV matmul also yields the softmax
    denominators (row 64 of OT).
  - scores (transposed layout) ST[k, q] = KT_h.T @ QT_h, exp on ACT,
    OT[65, q] += [V_h|1].T @ PT accumulated over k-tiles.
  - out projection per s-tile with both heads row-packed on the PE array,
    combined with per-partition 1/Z scaling on DVE.
"""

import numpy as np

import concourse.bass as bass
import concourse.tile as tile
from concourse import bacc, mybir
from concourse.bass import ds, ts
from concourse.bass_utils import run_bass_kernel_spmd
from concourse.masks import make_identity

B = 2
S = 2048
D = 1024
H = 16
DK = 64
N_CORES = 8
H_LOC = H // N_CORES          # 2 heads per core
D2 = H_LOC * DK               # 128 projected dims per core
DP = 1152                     # padded contraction dim: 1024 + bias row + pad
NDT = DP // 128               # 9 contraction tiles
BS = B * S                    # 4096
QC = 1024                     # query chunk
NKT = S // 128                # 16 key tiles per batch
FP = mybir.dt.float32
EXP = mybir.ActivationFunctionType.Exp
MUL = mybir.AluOpType.mult
ADD = mybir.AluOpType.add


def emit(tc: tile.TileContext, xt, wq, wk, wv, wo, y, taps=None):
    nc = tc.nc
    import contextlib

    ctx = contextlib.ExitStack()
    const_pool = ctx.enter_context(tc.tile_pool(name="const", bufs=1))
    wpool = ctx.enter_context(tc.tile_pool(name="w", bufs=1))
    xt_pool = ctx.enter_context(tc.tile_pool(name="xtp", bufs=9))
    proj_pool = ctx.enter_context(tc.tile_pool(name="proj", bufs=2))
    vsb_pool = ctx.enter_context(tc.tile_pool(name="vsbp", bufs=2))
    pt_pool = ctx.enter_context(tc.tile_pool(name="ptp", bufs=3))
    osb_pool = ctx.enter_context(tc.tile_pool(name="osbp", bufs=2))
    z_pool = ctx.enter_context(tc.tile_pool(name="zp", bufs=4))
    ysb_pool = ctx.enter_context(tc.tile_pool(name="ysbp", bufs=3))
    ps_pool = ctx.enter_context(tc.tile_pool(name="ps", bufs=3, space="PSUM"))
    ot_pool = ctx.enter_context(tc.tile_pool(name="otp", bufs=1, space="PSUM"))

    ident = const_pool.tile([128, 128], FP, tag="ident")
    make_identity(nc, ident[:])

    w_sb = {}
    for nm, wd in (("q", wq), ("k", wk), ("v", wv)):
        t = wpool.tile([128, NDT, D2], FP, tag=f"w{nm}", name=f"w{nm}sb")
        nc.sync.dma_start(t[:], wd.rearrange("(t p) m -> p t m", p=128))
        w_sb[nm] = t
    wo_sb = wpool.tile([D2, D], FP, tag="wo", name="wosb")
    nc.sync.dma_start(wo_sb[:], wo[:, :])

    for b in range(B):
        # ---- load x^T tiles for this batch ----
        xts = []
        for t in range(NDT):
            xtile = xt_pool.tile([128, S], FP, tag="xt", name=f"xt{b}_{t}")
            nc.sync.dma_start(xtile[:], xt[ts(t, 128), ds(b * S, S)])
            xts.append(xtile)

        # ---- projections: QT/KT/VT [128, S] ----
        proj_sb = {}
        for nm in ("q", "k", "v"):
            psb = proj_pool.tile([D2, S], FP, tag=f"p{nm}", name=f"p{nm}{b}")
            for ch in range(S // 512):
                acc = ps_pool.tile([128, QC], FP, tag="ps", name=f"acc{nm}{b}_{ch}")
                for t in range(NDT):
                    nc.tensor.matmul(
                        acc[:, 0:512],
                        w_sb[nm][:, t, :],
                        xts[t][:, ts(ch, 512)],
                        start=(t == 0),
                        stop=(t == NDT - 1),
                    )
                nc.vector.tensor_copy(psb[:, ts(ch, 512)], acc[:, 0:512])
            proj_sb[nm] = psb
        qt_sb, kt_sb, vt_sb = proj_sb["q"], proj_sb["k"], proj_sb["v"]

        if taps is not None and b == 0:
            nc.sync.dma_start(taps["qt"][:, :], qt_sb[:])
            nc.sync.dma_start(taps["kt"][:, :], kt_sb[:])
            nc.sync.dma_start(taps["vt"][:, :], vt_sb[:])

        # ---- V -> natural [k, d] layout with ones columns ----
        vsb = vsb_pool.tile([128, NKT * 130], FP, tag="vsb", name=f"vsb{b}")
        for k in range(NKT):
            tp = ps_pool.tile([128, 128], FP, tag="ps", name=f"vtr{b}_{k}")
            nc.tensor.transpose(tp[:], vt_sb[:, ts(k, 128)], ident[:])
            nc.vector.tensor_copy(vsb[:, ds(k * 130, 64)], tp[:, 0:64])
            nc.vector.tensor_copy(vsb[:, ds(k * 130 + 65, 64)], tp[:, 64:128])
            nc.gpsimd.memset(vsb[:, ds(k * 130 + 64, 1)], 1.0)
            nc.gpsimd.memset(vsb[:, ds(k * 130 + 129, 1)], 1.0)

        if taps is not None and b == 0:
            nc.sync.dma_start(taps["vsb"][:, :], vsb[:])

        # ---- attention ----
        for qc in range(S // QC):
            osb = osb_pool.tile([D2, QC], FP, tag="osb", name=f"osb{b}_{qc}")
            zr = {}
            for h in range(H_LOC):
                hsl = slice(h * DK, (h + 1) * DK)
                ot = ot_pool.tile([65, QC], FP, tag="ot", name=f"ot{b}_{qc}_{h}")
                for k in range(NKT):
                    st = ps_pool.tile([128, QC], FP, tag="ps", name=f"st{b}{qc}{h}_{k}")
                    for hf in range(QC // 512):
                        nc.tensor.matmul(
                            st[:, ts(hf, 512)],
                            kt_sb[hsl, ts(k, 128)],
                            qt_sb[hsl, ds(qc * QC + hf * 512, 512)],
                            start=True,
                            stop=True,
                        )
                    ptile = pt_pool.tile([128, QC], FP, tag="pt", name=f"pt{b}{qc}{h}_{k}")
                    nc.scalar.activation(ptile[:], st[:], EXP)
                    if taps is not None and b == 0 and qc == 0 and h == 0 and k < 2:
                        nc.sync.dma_start(taps[f"pt{k}"][:, :], ptile[:])
                    for hf in range(QC // 512):
                        nc.tensor.matmul(
                            ot[:, ts(hf, 512)],
                            vsb[:, ds(k * 130 + h * 65, 65)],
                            ptile[:, ts(hf, 512)],
                            start=(k == 0),
                            stop=(k == NKT - 1),
                        )
                # extract O (rows 0..63) and Z (row 64)
                nc.vector.tensor_copy(osb[hsl, :], ot[0:64, :])
                zrow = z_pool.tile([1, QC], FP, tag="zrow", name=f"zr{b}{qc}{h}")
                nc.vector.tensor_copy(zrow[:], ot[64:65, :])
                ztp = ps_pool.tile([128, QC // 128], FP, tag="ps", name=f"zt{b}{qc}{h}")
                for t in range(QC // 128):
                    nc.tensor.transpose(
                        ztp[:, ds(t, 1)], zrow[0:1, ts(t, 128)], ident[0:1, 0:1]
                    )
                zcol = z_pool.tile([128, QC // 128], FP, tag="zcol", name=f"zc{b}{qc}{h}")
                nc.vector.tensor_copy(zcol[:], ztp[:])
                zrec = z_pool.tile([128, QC // 128], FP, tag=f"zrec{h}", name=f"zi{b}{qc}{h}")
                nc.vector.reciprocal(zrec[:], zcol[:])
                zr[h] = zrec
                if taps is not None and b == 0 and qc == 0 and h == 0:
                    nc.sync.dma_start(taps["osb0"][:, :], osb[:])
                    nc.sync.dma_start(taps["zrow"][:, :], zrow[:])
                    nc.sync.dma_start(taps["zcol"][:, :], zcol[:])

            # ---- output projection (partial, this core's heads) ----
            for st_i in range(QC // 128):
                ysb = ysb_pool.tile([128, D], FP, tag="ysb", name=f"y{b}{qc}{st_i}")
                for e in range(D // 512):
                    yp = ps_pool.tile([128, QC], FP, tag="ps", name=f"yp{b}{qc}{st_i}{e}")
                    nc.tensor.matmul(
                        yp[:, 0:512],
                        osb[0:64, ts(st_i, 128)],
                        wo_sb[0:64, ts(e, 512)],
                        start=True,
                        stop=True,
                    )
                    nc.tensor.matmul(
                        yp[:, 512:1024],
                        osb[64:128, ts(st_i, 128)],
                        wo_sb[64:128, ts(e, 512)],
                        start=True,
                        stop=True,
                    )
                    nc.vector.tensor_scalar_mul(
                        ysb[:, ts(e, 512)], yp[:, 0:512], zr[0][:, ds(st_i, 1)]
                    )
                    nc.vector.scalar_tensor_tensor(
                        ysb[:, ts(e, 512)],
                        yp[:, 512:1024],
                        zr[1][:, ds(st_i, 1)],
                        ysb[:, ts(e, 512)],
                        op0=MUL,
                        op1=ADD,
                    )
                row = b * S + qc * QC + st_i * 128
                nc.sync.dma_start(y[ds(row, 128), :], ysb[:])

    ctx.close()


def build_nc(debug: bool = False, with_taps: bool = False):
    nc = bacc.Bacc(
        "TRN2", target_bir_lowering=False, debug=debug, num_devices=N_CORES
    )
    xt = nc.dram_tensor("xt", [DP, BS], FP, kind="ExternalInput").ap()
    wq = nc.dram_tensor("wq", [DP, D2], FP, kind="ExternalInput").ap()
    wk = nc.dram_tensor("wk", [DP, D2], FP, kind="ExternalInput").ap()
    wv = nc.dram_tensor("wv", [DP, D2], FP, kind="ExternalInput").ap()
    wo = nc.dram_tensor("wo", [D2, D], FP, kind="ExternalInput").ap()
    y = nc.dram_tensor("y", [BS, D], FP, kind="ExternalOutput").ap()
    taps = None
    if with_taps:
        shapes = {
            "qt": [D2, S], "kt": [D2, S], "vt": [D2, S],
            "vsb": [128, NKT * 130], "pt0": [128, QC], "pt1": [128, QC],
            "osb0": [D2, QC], "zrow": [1, QC], "zcol": [128, QC // 128],
        }
        taps = {
            nm: nc.dram_tensor(f"tap_{nm}", sh, FP, kind="ExternalOutput").ap()
            for nm, sh in shapes.items()
        }
    with tile.TileContext(nc) as tc:
        emit(tc, xt, wq, wk, wv, wo, y, taps=taps)
    nc.compile()
    return nc


def make_in_maps(x, Wq, bq, Wk, bk, Wv, bv, Wo):
    x = np.asarray(x, np.float32)
    xf = np.ascontiguousarray(x.reshape(BS, D))
    xt_aug = np.zeros((DP, BS), np.float32)
    xt_aug[:D] = xf.T
    xt_aug[D] = 1.0

    def aug(W, b, scale):
        a = np.zeros((DP, D2), np.float32)
        a[:D] = np.asarray(W, np.float32) * scale
        a[D] = np.asarray(b, np.float32) * scale
        return a

    in_maps = []
    for c in range(N_CORES):
        cols = slice(c * D2, (c + 1) * D2)
        in_maps.append(
            {
                "xt": xt_aug,
                "wq": aug(np.asarray(Wq)[:, cols], np.asarray(bq)[cols], 0.125),
                "wk": aug(np.asarray(Wk)[:, cols], np.asarray(bk)[cols], 1.0),
                "wv": aug(np.asarray(Wv)[:, cols], np.asarray(bv)[cols], 1.0),
                "wo": np.ascontiguousarray(np.asarray(Wo, np.float32)[cols, :]),
            }
        )
    return in_maps


_NC = None


def _get_nc():
    global _NC
    if _NC is None:
        _NC = build_nc(debug=False)
    return _NC


def kernel(x, Wq, bq, Wk, bk, Wv, bv, Wo, bo):
    in_maps = make_in_maps(x, Wq, bq, Wk, bk, Wv, bv, Wo)
    nc = _get_nc()
    res = run_bass_kernel_spmd(nc, in_maps, list(range(N_CORES))).results
    y = res[0]["y"].copy()
    for c in range(1, N_CORES):
        y += res[c]["y"]
    y += np.asarray(bo, np.float32)
    return y.reshape(B, S, D).astype(np.float32)


# revision 16
# speedup vs baseline: 1.6902x; 1.6902x over previous
"""Multi-head attention (B=2, S=2048, D=1024, H=16) on 8 TRN2 NeuronCores.

Sharding: tensor-parallel over heads. Each core owns 2 heads: Wq/Wk/Wv are
split column-wise (128 cols per core), Wo row-wise (128 rows per core). Each
core computes its heads' attention and a partial output projection
y_c = O_c @ Wo_c; the host sums the 8 partials and adds bo.

Per-core device pipeline (all fp32):
  - host passes xT_aug [1152, 4096]: x.T with a ones row (row 1024) so the
    projection biases ride in the matmul as an extra contraction row; rows
    1025.. are zero padding to a multiple of 128.
  - QT/KT/VT projections: out[d2=128, s] = Wq_aug.T @ xT_aug per 512-col
    chunk, accumulating 9 K-tiles in PSUM. Scale 1/sqrt(dk) is folded into
    Wq/bq on the host.
  - V transposed on PE (identity matmul) to natural [k, d] layout with a
    ones column appended per head -> PV matmul also yields the softmax
    denominators (row 64 of OT).
  - scores (transposed layout) ST[k, q] = KT_h.T @ QT_h, exp on ACT,
    OT[65, q] += [V_h|1].T @ PT accumulated over k-tiles.
  - out projection per s-tile with both heads row-packed on the PE array,
    combined with per-partition 1/Z scaling on DVE.
"""

import numpy as np

import concourse.bass as bass
import concourse.tile as tile
from concourse import bacc, mybir
from concourse.bass import ds, ts
from concourse.bass_utils import run_bass_kernel_spmd
from concourse.masks import make_identity

B = 2
S = 2048
D = 1024
H = 16
DK = 64
N_CORES = 8
H_LOC = H // N_CORES          # 2 heads per core
D2 = H_LOC * DK               # 128 projected dims per core
DP = 1152                     # padded contraction dim: 1024 + bias row + pad
NDT = DP // 128               # 9 contraction tiles
BS = B * S                    # 4096
QC = 1024                     # query chunk
NKT = S // 128                # 16 key tiles per batch
FP = mybir.dt.float32
FPR = mybir.dt.float32r


def round_fp32r(a):
    """Round fp32 array to fp32r (13-bit mantissa) as the PE consumes it."""
    b = np.ascontiguousarray(a, np.float32).view(np.uint32)
    return ((b + 0x800) & 0xFFFFF000).view(np.float32)

EXP = mybir.ActivationFunctionType.Exp
MUL = mybir.AluOpType.mult
ADD = mybir.AluOpType.add


def emit(tc: tile.TileContext, xt, wq, wk, wv, wo, y, taps=None):
    nc = tc.nc
    import contextlib

    ctx = contextlib.ExitStack()
    const_pool = ctx.enter_context(tc.tile_pool(name="const", bufs=1))
    wpool = ctx.enter_context(tc.tile_pool(name="w", bufs=1))
    xt_pool = ctx.enter_context(tc.tile_pool(name="xtp", bufs=9))
    proj_pool = ctx.enter_context(tc.tile_pool(name="proj", bufs=2))
    vsb_pool = ctx.enter_context(tc.tile_pool(name="vsbp", bufs=2))
    pt_pool = ctx.enter_context(tc.tile_pool(name="ptp", bufs=3))
    osb_pool = ctx.enter_context(tc.tile_pool(name="osbp", bufs=2))
    z_pool = ctx.enter_context(tc.tile_pool(name="zp", bufs=4))
    ysb_pool = ctx.enter_context(tc.tile_pool(name="ysbp", bufs=3))
    ps_pool = ctx.enter_context(tc.tile_pool(name="ps", bufs=3, space="PSUM"))
    ot_pool = ctx.enter_context(tc.tile_pool(name="otp", bufs=1, space="PSUM"))

    ident = const_pool.tile([128, 128], FP, tag="ident")
    make_identity(nc, ident[:])
    ones32 = const_pool.tile([128, 2 * NKT], FP, tag="ones32")
    nc.gpsimd.memset(ones32[:], 1.0)

    w_sb = {}
    for nm, wd in (("q", wq), ("k", wk), ("v", wv)):
        t = wpool.tile([128, NDT, D2], FPR, tag=f"w{nm}", name=f"w{nm}sb")
        nc.sync.dma_start(t[:], wd.rearrange("(t p) m -> p t m", p=128))
        w_sb[nm] = t
    wo_sb = wpool.tile([D2, D], FPR, tag="wo", name="wosb")
    nc.sync.dma_start(wo_sb[:], wo[:, :])

    for b in range(B):
        # ---- load x^T tiles for this batch ----
        xts = []
        for t in range(NDT):
            xtile = xt_pool.tile([128, S], FPR, tag="xt", name=f"xt{b}_{t}")
            nc.sync.dma_start(xtile[:], xt[ts(t, 128), ds(b * S, S)])
            xts.append(xtile)

        # ---- projections: QT/KT/VT [128, S] ----
        proj_sb = {}
        for nm in ("q", "k", "v"):
            psb = proj_pool.tile([D2, S], FPR if nm != "v" else FP, tag=f"p{nm}", name=f"p{nm}{b}")
            for ch in range(S // 512):
                acc = ps_pool.tile([128, QC], FP, tag="ps", name=f"acc{nm}{b}_{ch}")
                for t in range(NDT):
                    nc.tensor.matmul(
                        acc[:, 0:512],
                        w_sb[nm][:, t, :],
                        xts[t][:, ts(ch, 512)],
                        start=(t == 0),
                        stop=(t == NDT - 1),
                    )
                nc.vector.tensor_copy(psb[:, ts(ch, 512)], acc[:, 0:512])
            proj_sb[nm] = psb
        qt_sb, kt_sb, vt_sb = proj_sb["q"], proj_sb["k"], proj_sb["v"]

        if taps is not None and b == 0:
            nc.sync.dma_start(taps["qt"][:, :], qt_sb[:])
            nc.sync.dma_start(taps["kt"][:, :], kt_sb[:])
            nc.sync.dma_start(taps["vt"][:, :], vt_sb[:])

        # ---- V -> natural [k, d] layout with ones columns ----
        vsb = vsb_pool.tile([128, NKT * 130], FPR, tag="vsb", name=f"vsb{b}")
        for k in range(NKT):
            tp = ps_pool.tile([128, 128], FP, tag="ps", name=f"vtr{b}_{k}")
            nc.tensor.transpose(tp[:], vt_sb[:, ts(k, 128)], ident[:])
            nc.vector.tensor_copy(vsb[:, ds(k * 130, 64)], tp[:, 0:64])
            nc.vector.tensor_copy(vsb[:, ds(k * 130 + 65, 64)], tp[:, 64:128])
        # ones columns sit at free offset 64 + 65*j (both per 130-block)
        nc.vector.tensor_copy(
            vsb.rearrange("p (j c) -> p j c", c=65)[:, :, 64:65],
            ones32.rearrange("p (j c) -> p j c", c=1),
        )

        if taps is not None and b == 0:
            nc.sync.dma_start(taps["vsb"][:, :], vsb[:])

        # ---- attention ----
        for qc in range(S // QC):
            osb = osb_pool.tile([D2, QC], FPR, tag="osb", name=f"osb{b}_{qc}")
            zr = {}
            for h in range(H_LOC):
                hsl = slice(h * DK, (h + 1) * DK)
                ot = ot_pool.tile([65, QC], FP, tag="ot", name=f"ot{b}_{qc}_{h}")
                for k in range(NKT):
                    st = ps_pool.tile([128, QC], FP, tag="ps", name=f"st{b}{qc}{h}_{k}")
                    for hf in range(QC // 512):
                        nc.tensor.matmul(
                            st[:, ts(hf, 512)],
                            kt_sb[hsl, ts(k, 128)],
                            qt_sb[hsl, ds(qc * QC + hf * 512, 512)],
                            start=True,
                            stop=True,
                        )
                    ptile = pt_pool.tile([128, QC], FPR, tag="pt", name=f"pt{b}{qc}{h}_{k}")
                    nc.scalar.activation(ptile[:], st[:], EXP)
                    if taps is not None and b == 0 and qc == 0 and h == 0 and k < 2:
                        nc.sync.dma_start(taps[f"pt{k}"][:, :], ptile[:])
                    for hf in range(QC // 512):
                        nc.tensor.matmul(
                            ot[:, ts(hf, 512)],
                            vsb[:, ds(k * 130 + h * 65, 65)],
                            ptile[:, ts(hf, 512)],
                            start=(k == 0),
                            stop=(k == NKT - 1),
                        )
                # extract O (rows 0..63) and Z (row 64)
                nc.vector.tensor_copy(osb[hsl, :], ot[0:64, :])
                zrow = z_pool.tile([1, QC], FP, tag="zrow", name=f"zr{b}{qc}{h}")
                nc.vector.tensor_copy(zrow[:], ot[64:65, :])
                ztp = ps_pool.tile([128, QC // 128], FP, tag="ps", name=f"zt{b}{qc}{h}")
                for t in range(QC // 128):
                    nc.tensor.transpose(
                        ztp[:, ds(t, 1)], zrow[0:1, ts(t, 128)], ident[0:1, 0:1]
                    )
                zcol = z_pool.tile([128, QC // 128], FP, tag="zcol", name=f"zc{b}{qc}{h}")
                nc.vector.tensor_copy(zcol[:], ztp[:])
                zrec = z_pool.tile([128, QC // 128], FP, tag=f"zrec{h}", name=f"zi{b}{qc}{h}")
                nc.vector.reciprocal(zrec[:], zcol[:])
                zr[h] = zrec
                if taps is not None and b == 0 and qc == 0 and h == 0:
                    nc.sync.dma_start(taps["osb0"][:, :], osb[:])
                    nc.sync.dma_start(taps["zrow"][:, :], zrow[:])
                    nc.sync.dma_start(taps["zcol"][:, :], zcol[:])

            # ---- output projection (partial, this core's heads) ----
            for st_i in range(QC // 128):
                ysb = ysb_pool.tile([128, D], FP, tag="ysb", name=f"y{b}{qc}{st_i}")
                for e in range(D // 512):
                    yp = ps_pool.tile([128, QC], FP, tag="ps", name=f"yp{b}{qc}{st_i}{e}")
                    nc.tensor.matmul(
                        yp[:, 0:512],
                        osb[0:64, ts(st_i, 128)],
                        wo_sb[0:64, ts(e, 512)],
                        start=True,
                        stop=True,
                    )
                    nc.tensor.matmul(
                        yp[:, 512:1024],
                        osb[64:128, ts(st_i, 128)],
                        wo_sb[64:128, ts(e, 512)],
                        start=True,
                        stop=True,
                    )
                    nc.vector.tensor_scalar_mul(
                        ysb[:, ts(e, 512)], yp[:, 0:512], zr[0][:, ds(st_i, 1)]
                    )
                    nc.vector.scalar_tensor_tensor(
                        ysb[:, ts(e, 512)],
                        yp[:, 512:1024],
                        zr[1][:, ds(st_i, 1)],
                        ysb[:, ts(e, 512)],
                        op0=MUL,
                        op1=ADD,
                    )
                row = b * S + qc * QC + st_i * 128
                nc.sync.dma_start(y[ds(row, 128), :], ysb[:])

    ctx.close()


def build_nc(debug: bool = False, with_taps: bool = False):
    nc = bacc.Bacc(
        "TRN2", target_bir_lowering=False, debug=debug, num_devices=N_CORES
    )
    xt = nc.dram_tensor("xt", [DP, BS], FPR, kind="ExternalInput").ap()
    wq = nc.dram_tensor("wq", [DP, D2], FPR, kind="ExternalInput").ap()
    wk = nc.dram_tensor("wk", [DP, D2], FPR, kind="ExternalInput").ap()
    wv = nc.dram_tensor("wv", [DP, D2], FPR, kind="ExternalInput").ap()
    wo = nc.dram_tensor("wo", [D2, D], FPR, kind="ExternalInput").ap()
    y = nc.dram_tensor("y", [BS, D], FP, kind="ExternalOutput").ap()
    taps = None
    if with_taps:
        shapes = {
            "qt": [D2, S], "kt": [D2, S], "vt": [D2, S],
            "vsb": [128, NKT * 130], "pt0": [128, QC], "pt1": [128, QC],
            "osb0": [D2, QC], "zrow": [1, QC], "zcol": [128, QC // 128],
        }
        taps = {
            nm: nc.dram_tensor(f"tap_{nm}", sh, FP, kind="ExternalOutput").ap()
            for nm, sh in shapes.items()
        }
    with tile.TileContext(nc) as tc:
        emit(tc, xt, wq, wk, wv, wo, y, taps=taps)
    nc.compile()
    return nc


def make_in_maps(x, Wq, bq, Wk, bk, Wv, bv, Wo):
    x = np.asarray(x, np.float32)
    xf = np.ascontiguousarray(x.reshape(BS, D))
    xt_aug = np.zeros((DP, BS), np.float32)
    xt_aug[:D] = xf.T
    xt_aug[D] = 1.0
    xt_aug = round_fp32r(xt_aug)

    def aug(W, b, scale):
        a = np.zeros((DP, D2), np.float32)
        a[:D] = np.asarray(W, np.float32) * scale
        a[D] = np.asarray(b, np.float32) * scale
        return round_fp32r(a)

    in_maps = []
    for c in range(N_CORES):
        cols = slice(c * D2, (c + 1) * D2)
        in_maps.append(
            {
                "xt": xt_aug,
                "wq": aug(np.asarray(Wq)[:, cols], np.asarray(bq)[cols], 0.125),
                "wk": aug(np.asarray(Wk)[:, cols], np.asarray(bk)[cols], 1.0),
                "wv": aug(np.asarray(Wv)[:, cols], np.asarray(bv)[cols], 1.0),
                "wo": round_fp32r(np.asarray(Wo, np.float32)[cols, :]),
            }
        )
    return in_maps


_NC = None


def _get_nc():
    global _NC
    if _NC is None:
        _NC = build_nc(debug=False)
    return _NC


def kernel(x, Wq, bq, Wk, bk, Wv, bv, Wo, bo):
    in_maps = make_in_maps(x, Wq, bq, Wk, bk, Wv, bv, Wo)
    nc = _get_nc()
    res = run_bass_kernel_spmd(nc, in_maps, list(range(N_CORES))).results
    y = res[0]["y"].copy()
    for c in range(1, N_CORES):
        y += res[c]["y"]
    y += np.asarray(bo, np.float32)
    return y.reshape(B, S, D).astype(np.float32)
